# revision 2
# baseline (speedup 1.0000x reference)
"""Trainium2 Bass kernel for CustomMultiheadAttention (linear attention with
low-rank QKV projections) — sequence-sharded, fp8 DoubleRow, v10.

Math (fp32 reference):
    q = elu(query @ Wq.T + q_up_b) + 1      with Wq = q_up_w @ q_down_w  [E,E]
    k = elu(key   @ Wk.T + k_up_b) + 1
    v =      value @ Wv.T (+ v_up_b: folded into the output bias on host)
    per head h (16 heads, head_dim 64):
        kv_h   = k_h^T v_h      (sum over ALL tokens)
        ksum_h = sum_t k_h[t]
        attn_h = (q_h kv_h) / (q_h . ksum_h)
    out = concat_h(attn_h) @ out_w.T + out_b + v_up_b @ out_w.T

Sharding: 8 cores = 4 batches x 2 sequence-halves (2048 tokens per core);
cross-core data is only the per-head kv/ksum accumulators (pairwise bf16
AllReduce overlapped with the q-projection matmuls).

v6 over v5: the four big projections (Q/K/V features + output) run as
fp8(e4m3) DoubleRow matmuls — both operands quantized with power-of-2
absmax scales on the host (x, W) or on-device (attn), two 128-row
contraction slices per PE instruction. Dequantization folds into ops the
pipeline already had (activation `scale`, DVE scalar ops). Host-simulated
max-rel-err ~9e-3 vs the fp32 reference (gate 2e-2).

v7 over v6 (v6 measured DVE/Scalar co-critical with the PE, causing HAM
oscillation: 105us throttled):
  - engine rebalance: v-dequant + kv accumulate + half the attn scaling +
    half the output dequant move to the (previously idle) GpSimd engine;
    feature-combine DVE ops take bf16 inputs/outputs for the 2x DVE mode.
  - attn scale: one scalar_tensor_tensor (num*S_ATTN)*rep replaces the
    scalar-engine reciprocal-broadcast copy + DVE multiply.
  - denominator reciprocal: reciprocal_approx_fast (5x faster, 18 good
    bits) pipelined one chunk ahead so it never gates the rep matmul; the
    replication matmul runs f32r so no bf16 cast is needed.
  - the AllReduce payload is fp8 (67 KB): kv absmax ~110 and ksum/32
    ~140 both sit inside e4m3 range with no extra scaling ops.
  - phase Q emission interleaves q-features with attention chunks under
    monotone tile_wait_until stamps so the in-order PE queue alternates
    AR-independent and AR-dependent work.

ksum rides the kv matmul as two extra (1/32)-columns so the packed payload
stays in one dtype; the 32x is restored in the head-mask constant.
"""

import numpy as np
import ml_dtypes
from ml_dtypes import bfloat16

import concourse.bass as bass  # noqa: F401
import concourse.mybir as mybir
import concourse.tile as tile
from concourse import bacc
from concourse.bass_utils import run_bass_kernel_spmd

F32 = mybir.dt.float32
F32R = mybir.dt.float32r
BF16 = mybir.dt.bfloat16
FP8 = mybir.dt.float8e4
AF = mybir.ActivationFunctionType
OP = mybir.AluOpType
DR = mybir.MatmulPerfMode.DoubleRow

P = 128           # partitions
E = 1024          # embed dim
NH = 16           # heads
S_LOC = 2048      # tokens per core
TC = 512          # token chunk
NCH = S_LOC // TC  # 4 chunks
NE = E // P       # 8 e-tiles
NE2 = NE // 2     # 4 DoubleRow e-pairs
NJ = E // P       # 8 j-tiles (2 heads each)
NJ2 = NJ // 2     # 4 DoubleRow j-pairs
NTS = TC // P     # 4 token subtiles per chunk
KW = P + 2        # kv block width: 128 v-cols + 2 (1/32)-cols (ksum/32)
AW = 66           # packed allreduce width/jt: two 64x64 diag blocks + ksum
S_ATTN = 1024.0   # attn fp8 quant scale (attn absmax ~0.02 -> ~22)
NCST = NJ * NH + NJ + 4  # rtm | bqt | scales

_CACHE = {}


def _build():
    nc = bacc.Bacc(None, target_bir_lowering=False, num_devices=8)

    # all inputs are pre-tiled on the host into device layout so every DMA
    # is one contiguous run per partition
    dp = nc.declare_dram_parameter
    xq = dp("xq", [NCH, P, NE, TC], FP8, isOutput=False)
    xk = dp("xk", [NCH, P, NE, TC], FP8, isOutput=False)
    xv = dp("xv", [NCH, P, NE, TC], FP8, isOutput=False)
    wq = dp("wq", [P, NE, E], FP8, isOutput=False)  # (q_up@q_down).T tiled
    wk = dp("wk", [2, P, NE, TC], FP8, isOutput=False)  # j-halves
    wv = dp("wv", [2, P, NE, TC], FP8, isOutput=False)
    wo = dp("wo", [P, NJ, E], FP8, isOutput=False)  # out_w.T tiled
    cst = dp("cst", [P, NCST], F32, isOutput=False)  # rtm|bqt|scales
    bkb = dp("bkb", [P, E], BF16, isOutput=False)    # k bias broadcast
    r8m = dp("r8m", [NH, E], BF16, isOutput=False)   # head-replication mask
    out_t = dp("out", [S_LOC, E], F32, isOutput=True)

    with tile.TileContext(nc) as tcx:
        from contextlib import ExitStack

        with ExitStack() as root:
            cpool = root.enter_context(tcx.tile_pool(name="consts", bufs=1))
            xqpool = root.enter_context(tcx.tile_pool(name="xqp", bufs=4))
            dram = root.enter_context(
                tcx.tile_pool(name="dram", bufs=1, space="DRAM"))
            csb = cpool.tile([P, NCST], F32)
            rt_sb = csb[:, 0:NJ * NH]              # 32*head-mask
            bqt_sb = csb[:, NJ * NH:NJ * NH + NJ]  # q bias, per-jt columns
            sc_kdq = csb[:, NCST - 4:NCST - 3]     # 1/(sxk*swk)
            sc_qdq = csb[:, NCST - 3:NCST - 2]     # 1/(sxq*swq)
            sc_vdq = csb[:, NCST - 2:NCST - 1]     # 1/(sxv*swv)
            sc_ob = csb[:, NCST - 1:NCST]          # 1/(S_ATTN*swo)
            bkb_sb = cpool.tile([P, E], BF16)
            r8_sb = cpool.tile([NH, E], BF16)
            abuf = cpool.tile([P, NJ, AW], FP8)    # packed allreduce payload
            ksr = cpool.tile([P, NJ], F32)         # reduced ksum/32
            kblk = cpool.tile([P, NJ * NH], BF16)  # ksum masked into head cols
            wq_sb = cpool.tile([P, NE, E], FP8)    # prefetched during phase KV
            wo_sb = cpool.tile([P, NJ, E], FP8)
            ab_in = dram.tile([P, NJ, AW], FP8)
            ab_out = dram.tile([P, NJ, AW], FP8)

            # ---------------- Phase KV ----------------
            with ExitStack() as ph:
                wpool = ph.enter_context(tcx.tile_pool(name="wkv", bufs=1))
                wk_h = [wpool.tile([P, NE, TC], FP8, name=f"wk{i}") for i in range(2)]
                wv_h = [wpool.tile([P, NE, TC], FP8, name=f"wv{i}") for i in range(2)]
                xkpool = ph.enter_context(tcx.tile_pool(name="xkp", bufs=2))
                xvpool = ph.enter_context(tcx.tile_pool(name="xvp", bufs=2))
                xk0t = xkpool.tile([P, NE, TC], FP8, tag="xk", name="xk")
                xv0t = xvpool.tile([P, NE, TC], FP8, tag="xv", name="xv")

                # startup-critical tiles spread across the 3 DMA queues; the
                # k/v feature loops run jh-outer so each weight j-half is
                # needed only after ~its queue position's worth of DMA
                nc.sync.dma_start(out=xk0t[:], in_=xk[0])
                nc.scalar.dma_start(out=wk_h[0][:], in_=wk[0])
                nc.gpsimd.dma_start(csb[:], cst[:])
                nc.gpsimd.dma_start(bkb_sb[:], bkb[:])
                nc.gpsimd.dma_start(wk_h[1][:], wk[1])
                nc.sync.dma_start(out=xv0t[:], in_=xv[0])
                nc.scalar.dma_start(out=wv_h[0][:], in_=wv[0])
                nc.scalar.dma_start(out=wq_sb[:], in_=wq[:])
                nc.gpsimd.dma_start(wv_h[1][:], wv[1])
                nc.gpsimd.dma_start(r8_sb[:], r8m[:])
                nc.gpsimd.dma_start(wo_sb[:], wo[:])

                fpool = ph.enter_context(tcx.tile_pool(name="fkv", bufs=1))
                tpool = ph.enter_context(tcx.tile_pool(name="tkv", bufs=3))
                psf = ph.enter_context(
                    tcx.tile_pool(name="psf", bufs=1, space="PSUM"))
                psk = ph.enter_context(
                    tcx.tile_pool(name="psk", bufs=1, space="PSUM"))

                kfeat = fpool.tile([P, NTS, E], BF16)
                vch = fpool.tile([P, NTS, NJ, KW], BF16)
                # (1/32)-columns for the fused ksum (keeps ksum/32 in the
                # fp8-range of the packed payload; kblk mask restores 32x)
                nc.gpsimd.memset(vch[:], 1.0 / 32.0)
                # kv PSUM accumulators live across the whole chunk loop
                pkv = [
                    psk.tile([P, 3 * KW], F32, tag="pkv0", name="pkv0"),
                    psk.tile([P, 3 * KW], F32, tag="pkv1", name="pkv1"),
                    psk.tile([P, 2 * KW], F32, tag="pkv2", name="pkv2"),
                ]

                for ci in range(NCH):
                    if ci == 0:
                        xkt, xvt = xk0t, xv0t
                    else:
                        xkt = xkpool.tile([P, NE, TC], FP8, tag="xk",
                                          name="xk")
                        xvt = xvpool.tile([P, NE, TC], FP8, tag="xv",
                                          name="xv")
                        nc.sync.dma_start(out=xkt[:], in_=xk[ci])
                        nc.sync.dma_start(out=xvt[:], in_=xv[ci])

                    # k and v features, software-pipelined: the 4-stage k
                    # chain (dequant+bias -> relu/exp -> combine) fully
                    # serializes if emitted per-tile (in-order engine
                    # queues), so the combine lags one tile and v tiles
                    # interleave so the PE never outruns the consumers.
                    # chunk 0 runs k-first (the v weights are still in
                    # flight on the gpsimd DMA queue at ~12us).
                    if ci == 0:
                        order = ([("k", jh, tb) for jh in range(2)
                                  for tb in range(NTS)] +
                                 [("v", jh, tb) for jh in range(2)
                                  for tb in range(NTS)])
                    else:
                        order = [(kind, jh, tb) for jh in range(2)
                                 for tb in range(NTS)
                                 for kind in ("k", "v")]
                    pending = None
                    nv = 0
                    for kind, jh, tb in order:
                        if kind == "k":
                            js = slice(TC * jh, TC * (jh + 1))
                            pu = psf.tile([P, TC], F32, tag=f"k{tb % 2}",
                                          name="pu")
                            for e in range(NE2):
                                nc.tensor.matmul(
                                    pu[:],
                                    xkt[:, 2 * e:2 * e + 2, P * tb:P * (tb + 1)],
                                    wk_h[jh][:, 2 * e:2 * e + 2, :],
                                    start=(e == 0), stop=(e == NE2 - 1),
                                    perf_mode=DR,
                                )
                            u = tpool.tile([P, TC], F32, tag="u", name="u")
                            r = tpool.tile([P, TC], BF16, tag="r", name="r")
                            ex = tpool.tile([P, TC], BF16, tag="ex",
                                            name="ex")
                            nc.vector.scalar_tensor_tensor(
                                u[:], pu[:], sc_kdq, bkb_sb[:, js],
                                op0=OP.mult, op1=OP.add)
                            nc.scalar.activation(r[:], u[:], AF.Relu)
                            nc.scalar.activation(ex[:], u[:], AF.Exp)
                            if pending is not None:
                                ptb, pjs, pex, pr = pending
                                nc.vector.scalar_tensor_tensor(
                                    kfeat[:, ptb, pjs], pex[:], 1.0, pr[:],
                                    op0=OP.min, op1=OP.add)
                            pending = (tb, js, ex, r)
                        else:
                            pv = psf.tile([P, TC], F32, tag=f"v{tb % 2}",
                                          name="pv")
                            for e in range(NE2):
                                nc.tensor.matmul(
                                    pv[:],
                                    xvt[:, 2 * e:2 * e + 2, P * tb:P * (tb + 1)],
                                    wv_h[jh][:, 2 * e:2 * e + 2, :],
                                    start=(e == 0), stop=(e == NE2 - 1),
                                    perf_mode=DR,
                                )
                            vdst = vch[:, tb, 4 * jh:4 * (jh + 1), 0:P]
                            if nv % 2 == 0:
                                nc.scalar.activation(vdst, pv[:], AF.Copy,
                                                     scale=sc_vdq)
                            else:
                                nc.vector.tensor_scalar(
                                    vdst, pv[:], sc_vdq, None, op0=OP.mult)
                            nv += 1
                    ptb, pjs, pex, pr = pending
                    nc.vector.scalar_tensor_tensor(
                        kfeat[:, ptb, pjs], pex[:], 1.0, pr[:],
                        op0=OP.min, op1=OP.add)

                    # kv[j1, (j2|ones/32)] += sum_t kfeat[t,j1] vch[t,j1-tile]
                    # accumulated in PSUM across all four chunks. start=True
                    # zeroes a whole 2KB zero-region (= bank), so only the
                    # first j-tile of each bank starts; its siblings write
                    # into the pending-zero region with start=False.
                    # On the last chunk each j-tile's payload pack follows
                    # its final matmul so the pack overlaps the kv stream.
                    for jt in range(NJ):
                        dst = pkv[jt // 3][:, KW * (jt % 3):KW * (jt % 3 + 1)]
                        jb = slice(P * jt, P * (jt + 1))
                        for ts in range(NTS):
                            nc.tensor.matmul(
                                dst, kfeat[:, ts, jb], vch[:, ts, jt, :],
                                start=(ci == 0 and ts == 0 and jt % 3 == 0),
                                stop=(ci == NCH - 1 and ts == NTS - 1),
                                skip_group_check=True,
                            )
                        if ci == NCH - 1:
                            if jt % 2 == 0:
                                nc.vector.tensor_copy(
                                    abuf[0:64, jt, 0:64], dst[0:64, 0:64])
                                nc.vector.tensor_copy(
                                    abuf[64:P, jt, 0:64], dst[64:P, 64:P])
                                nc.vector.tensor_copy(
                                    abuf[:, jt, 64:AW], dst[:, P:P + 2])
                            else:
                                nc.scalar.activation(
                                    abuf[0:64, jt, 0:64], dst[0:64, 0:64],
                                    AF.Copy)
                                nc.scalar.activation(
                                    abuf[64:P, jt, 0:64], dst[64:P, 64:P],
                                    AF.Copy)
                                nc.scalar.activation(
                                    abuf[:, jt, 64:AW], dst[:, P:P + 2],
                                    AF.Copy)

                # prefetch the q chunks (sync queue, after the kv-side x)
                xqts = []
                for qi in range(NCH):
                    xqt_p = xqpool.tile([P, NE, TC], FP8, tag="xq",
                                        name="xqt_p")
                    nc.sync.dma_start(out=xqt_p[:], in_=xq[qi])
                    xqts.append(xqt_p)

                # (the AR payload pack is interleaved into the last chunk's
                # kv matmul stream above)

            # pairwise AllReduce of (kv2, ksum); overlaps phase-Q matmuls
            nc.gpsimd.dma_start(ab_in[:], abuf[:])
            nc.gpsimd.collective_compute(
                "AllReduce",
                OP.add,
                replica_groups=[[0, 1], [2, 3], [4, 5], [6, 7]],
                ins=[ab_in[:].opt()],
                outs=[ab_out[:].opt()],
            )
            nc.gpsimd.dma_start(abuf[:], ab_out[:])
            # everything downstream of the AllReduce is emitted under a late
            # scheduling timestamp: the Tile scheduler's cost model treats
            # the collective as near-instant, so without this it interleaves
            # AR-dependent ops into the scalar/vector queues between Q1
            # chunks, and the in-order engine queues then head-of-line block
            # on the AR for tens of us
            with tcx.tile_wait_until(0.5):
                # no kv unpack: the num matmul reads the packed payload
                # directly as two 64-partition block matmuls. Only ksum
                # needs extracting (DVE — shortest path to the first den).
                nc.vector.tensor_copy(ksr[:], abuf[:, :, 64:65])

                # kblk[p, jt, h] = 32*ksum[dim]/32 if dim in head h else 0
                for jt in range(NJ):
                    hs = slice(NH * jt, NH * (jt + 1))
                    nc.vector.tensor_scalar(
                        kblk[:, hs], rt_sb[:, hs],
                        ksr[:, jt:jt + 1], None, op0=OP.mult)

            # ---------------- Phase Q + attention + output ----------------
            with ExitStack() as ph:
                qpool = ph.enter_context(tcx.tile_pool(name="qf", bufs=4))
                tpool = ph.enter_context(tcx.tile_pool(name="tq", bufs=2))
                apool = ph.enter_context(tcx.tile_pool(name="attn", bufs=2))
                qspool = ph.enter_context(tcx.tile_pool(name="qs", bufs=2))
                opool = ph.enter_context(tcx.tile_pool(name="osb", bufs=2))
                psq = ph.enter_context(
                    tcx.tile_pool(name="psq", bufs=2, space="PSUM"))
                psd = ph.enter_context(
                    tcx.tile_pool(name="psd", bufs=1, space="PSUM"))
                psn = ph.enter_context(
                    tcx.tile_pool(name="psn", bufs=1, space="PSUM"))
                psr = ph.enter_context(
                    tcx.tile_pool(name="psr", bufs=1, space="PSUM"))
                pso = ph.enter_context(
                    tcx.tile_pool(name="pso", bufs=1, space="PSUM"))

                def emit_q(ci, xqt):
                    """q features for chunk ci, [j-part, t] layout."""
                    qT = qpool.tile([P, NJ, TC], BF16, tag="qT", name="qT")
                    for jt in range(NJ):
                        pq = psq.tile([P, TC], F32, tag="pq", name="pq")
                        for e in range(NE2):
                            nc.tensor.matmul(
                                pq[:],
                                wq_sb[:, 2 * e:2 * e + 2, P * jt:P * (jt + 1)],
                                xqt[:, 2 * e:2 * e + 2, :],
                                start=(e == 0), stop=(e == NE2 - 1),
                                perf_mode=DR,
                            )
                        bq_ap = bqt_sb[:, jt:jt + 1]
                        r = tpool.tile([P, TC], BF16, tag="qr", name="qr")
                        ex = tpool.tile([P, TC], BF16, tag="qe", name="qe")
                        nc.scalar.activation(r[:], pq[:], AF.Relu,
                                             bias=bq_ap, scale=sc_qdq)
                        nc.scalar.activation(ex[:], pq[:], AF.Exp,
                                             bias=bq_ap, scale=sc_qdq)
                        # all-bf16 operands -> 2x DVE mode
                        nc.vector.scalar_tensor_tensor(
                            qT[:, jt, :], ex[:], 1.0, r[:],
                            op0=OP.min, op1=OP.add,
                        )
                    return qT

                def emit_den(ci, qT):
                    """denominator reciprocal for chunk ci; emitted one chunk
                    ahead of its attention so the reciprocal hides under the
                    previous chunk's out-projection matmuls."""
                    pdn = psd.tile([NH, TC], F32, tag="pdn", name="pdn")
                    for jt in range(NJ):
                        nc.tensor.matmul(
                            pdn[:], kblk[:, NH * jt:NH * (jt + 1)],
                            qT[:, jt, :],
                            start=(jt == 0), stop=(jt == NJ - 1),
                        )
                    rcf = tpool.tile([NH, TC], F32, tag="rcf", name="rcf")
                    rcp = tpool.tile([NH, TC], BF16, tag="rcp", name="rcp")
                    # den >= ~1e5, far from the approx's undefined edge
                    # cases; 18 good bits vastly exceeds the 2e-2 gate
                    nc.vector.reciprocal_approx_fast(rcf[:], pdn[:])
                    nc.vector.tensor_copy(rcp[:], rcf[:])
                    return rcp

                def emit_attn(ci, qT, rcp):
                    """attn = (q * S_ATTN/den) @ kv: the reciprocal scales
                    qT BEFORE the kv matmul, so its PSUM result IS the
                    scaled attention — one DVE op + one fp8 copy per tile
                    instead of broadcast-copy + multiply."""
                    attn = apool.tile([P, NJ, TC], FP8, tag="attn",
                                      name="attn")
                    for jt in range(NJ):
                        prp = psr.tile([P, TC], F32, tag="prp", name="prp")
                        nc.tensor.matmul(
                            prp[:], r8_sb[:, P * jt:P * (jt + 1)], rcp[:],
                            start=True, stop=True,
                        )
                        qs = qspool.tile([P, TC], BF16, tag="qs", name="qs")
                        nc.vector.tensor_tensor(
                            qs[:], prp[:], qT[:, jt, :], op=OP.mult)
                        # block-diagonal kv straight from the packed AR
                        # payload: two 64-partition matmuls per j-tile
                        pnm = psn.tile([P, TC], F32, tag=f"pnm{jt % 2}",
                                       name="pnm")
                        nc.tensor.matmul(
                            pnm[0:64, :], abuf[0:64, jt, 0:64], qs[0:64, :],
                            start=True, stop=True,
                        )
                        nc.tensor.matmul(
                            pnm[64:P, :], abuf[64:P, jt, 0:64], qs[64:P, :],
                            start=True, stop=True,
                        )
                        if jt % 2 == 0:
                            nc.scalar.activation(attn[:, jt, :], pnm[:],
                                                 AF.Copy)
                        else:
                            nc.vector.tensor_copy(attn[:, jt, :], pnm[:])
                    return attn

                def emit_out(ci, attn):
                    # out[t, o] = sum_j attn[j, t] wo[j, o]; the two o-halves
                    # share each stationary attn j-pair
                    for tb in range(NTS):
                        ob = opool.tile([P, 2, TC], F32, tag="ob", name="ob")
                        po = [pso.tile([P, TC], F32, tag=f"po{oh}",
                                       name="po") for oh in range(2)]
                        for j in range(NJ2):
                            for oh in range(2):
                                nc.tensor.matmul(
                                    po[oh][:],
                                    attn[:, 2 * j:2 * j + 2, P * tb:P * (tb + 1)],
                                    wo_sb[:, 2 * j:2 * j + 2, TC * oh:TC * (oh + 1)],
                                    start=(j == 0), stop=(j == NJ2 - 1),
                                    perf_mode=DR,
                                )
                        nc.scalar.activation(ob[:, 0, :], po[0][:], AF.Copy,
                                             scale=sc_ob)
                        nc.vector.tensor_scalar(
                            ob[:, 1, :], po[1][:], sc_ob, None, op0=OP.mult)
                        row0 = ci * TC + tb * P
                        if ci == NCH - 1 and tb == NTS - 1:
                            # quarter the final stores: the tail drains two
                            # queues in parallel
                            h = TC // 2
                            nc.sync.dma_start(
                                out=out_t[row0:row0 + P, 0:h],
                                in_=ob[:, 0, 0:h])
                            nc.scalar.dma_start(
                                out=out_t[row0:row0 + P, h:TC],
                                in_=ob[:, 0, h:TC])
                            nc.sync.dma_start(
                                out=out_t[row0:row0 + P, TC:TC + h],
                                in_=ob[:, 1, 0:h])
                            nc.scalar.dma_start(
                                out=out_t[row0:row0 + P, TC + h:E],
                                in_=ob[:, 1, h:TC])
                        else:
                            nc.sync.dma_start(out=out_t[row0:row0 + P, 0:TC],
                                              in_=ob[:, 0, :])
                            nc.scalar.dma_start(
                                out=out_t[row0:row0 + P, TC:E],
                                in_=ob[:, 1, :])

                # chunks 0/1 of the q features fill the AllReduce window;
                # the rest interleaves with AR-dependent attention work under
                # monotone wait stamps so the in-order PE queue alternates
                # scalar-bound q chunks with PE-bound attention chunks
                qTs = {ci: None for ci in range(NCH)}
                qTs[0] = emit_q(0, xqts[0])
                qTs[1] = emit_q(1, xqts[1])
                rcps = {}
                attns = {}
                ws = [0.500, 0.502, 0.504, 0.506, 0.508, 0.510, 0.512,
                      0.514, 0.516, 0.518, 0.520, 0.522, 0.524, 0.526]
                plan = [
                    ("den", 0), ("attn", 0), ("den", 1), ("out", 0),
                    ("q", 2), ("attn", 1), ("den", 2), ("out", 1),
                    ("q", 3), ("attn", 2), ("den", 3), ("out", 2),
                    ("attn", 3), ("out", 3),
                ]
                for stamp, (kind, ci) in zip(ws, plan):
                    with tcx.tile_wait_until(stamp):
                        if kind == "q":
                            qTs[ci] = emit_q(ci, xqts[ci])
                        elif kind == "den":
                            rcps[ci] = emit_den(ci, qTs[ci])
                        elif kind == "attn":
                            attns[ci] = emit_attn(ci, qTs[ci], rcps[ci])
                        else:
                            emit_out(ci, attns[ci])

    nc.compile()
    return nc


def _get_nc():
    if "nc" not in _CACHE:
        _CACHE["nc"] = _build()
    return _CACHE["nc"]


def _pow2_scale(a, target=128.0):
    return float(2.0 ** np.floor(np.log2(target / float(np.abs(a).max()))))


def _q8(a, s):
    return np.clip(a * s, -240.0, 240.0).astype(ml_dtypes.float8_e4m3)


def kernel(**inputs):
    query = np.asarray(inputs["query"], dtype=np.float32)
    key = np.asarray(inputs["key"], dtype=np.float32)
    value = np.asarray(inputs["value"], dtype=np.float32)

    f32 = np.float32
    Wq = (inputs["q_up_w"] @ inputs["q_down_w"]).astype(f32)   # [E, E]
    Wk = (inputs["k_up_w"] @ inputs["k_down_w"]).astype(f32)
    Wv = (inputs["v_up_w"] @ inputs["v_down_w"]).astype(f32)
    Wo = np.asarray(inputs["out_w"], f32)

    sxq, sxk, sxv = (_pow2_scale(a) for a in (query, key, value))
    swq, swk, swv, swo = (_pow2_scale(w) for w in (Wq, Wk, Wv, Wo))

    def tile_in(wT):
        # [E_in, N] -> [P, NE, N] with in-index = a*P + p
        return np.ascontiguousarray(wT.reshape(NE, P, -1).transpose(1, 0, 2))

    def tile_half(wT):
        # [E_in, E_out] -> [2, P, NE, TC] split on the out j-halves
        return np.ascontiguousarray(
            wT.reshape(NE, P, 2, TC).transpose(2, 1, 0, 3))

    com = {
        "wq": tile_in(_q8(Wq.T, swq)),
        "wk": tile_half(_q8(Wk.T, swk)),
        "wv": tile_half(_q8(Wv.T, swv)),
        "wo": tile_in(_q8(Wo.T, swo)),
    }
    # head masks: full[d, h] = 1 iff dim d belongs to head h; x32 restores
    # the (1/32) ksum-column scaling
    heads = np.arange(E) // 64
    full = (heads[:, None] == np.arange(NH)[None, :]).astype(f32)
    rtm = (32.0 * full).reshape(NJ, P, NH).transpose(1, 0, 2).reshape(
        P, NJ * NH)
    bqt = np.asarray(inputs["q_up_b"], f32).reshape(NJ, P).T
    scales = np.broadcast_to(np.array(
        [1.0 / (sxk * swk), 1.0 / (sxq * swq), 1.0 / (sxv * swv),
         1.0 / (S_ATTN * swo)], f32), (P, 4))
    com["cst"] = np.ascontiguousarray(
        np.concatenate([rtm, bqt, scales], axis=1), dtype=f32)
    com["bkb"] = np.ascontiguousarray(np.broadcast_to(
        np.asarray(inputs["k_up_b"], f32), (P, E))).astype(bfloat16)
    # replication mask carries the attn fp8 quant scale
    com["r8m"] = np.ascontiguousarray(S_ATTN * full.T).astype(bfloat16)

    in_maps = []
    for c in range(8):
        b, h = divmod(c, 2)
        ts = slice(h * S_LOC, (h + 1) * S_LOC)

        def tile_x(x, sx):
            # [S_LOC, E] -> [NCH, P, NE, TC]: chunk ci, partition p, e-tile a
            # holds x.T[a*P + p, ci*TC + t]
            return np.ascontiguousarray(
                _q8(x.T, sx).reshape(NE, P, NCH, TC).transpose(2, 1, 0, 3))

        im = {
            "xq": tile_x(query[b, ts], sxq),
            "xk": tile_x(key[b, ts], sxk),
            "xv": tile_x(value[b, ts], sxv),
        }
        im.update(com)
        in_maps.append(im)

    nc = _get_nc()
    # the first execution after a device wedge occasionally dies with
    # NRT_EXEC_UNIT_UNRECOVERABLE; a retry on a clean session recovers
    last_err = None
    for _attempt in range(3):
        try:
            res = run_bass_kernel_spmd(nc, in_maps, core_ids=list(range(8)),
                                       **_CACHE.get("run_kwargs", {}))
            last_err = None
            break
        except Exception as e:  # noqa: BLE001
            last_err = e
            import time
            time.sleep(10)
    if last_err is not None:
        raise last_err
    _CACHE["last_result"] = res

    # v bias passes through the attention average; fold it into the out bias
    out_b = np.asarray(inputs["out_b"], f32) + (
        Wo @ np.asarray(inputs["v_up_b"], f32))
    B = query.shape[0]
    out = np.empty((B, 2 * S_LOC, E), np.float32)
    for c in range(8):
        b, h = divmod(c, 2)
        out[b, h * S_LOC:(h + 1) * S_LOC] = res.results[c]["out"] + out_b
    return out
